# revision 10
# baseline (speedup 1.0000x reference)
# Trainium2 Bass kernel for nn_CrossAttention (B=2, Nq=4096, Nk=2048, D=128,
# Dv=768, H=4, hd=32).
#
# Sharding: data-parallel over (B x Nq-blocks): core c handles batch c//4,
# query rows (c%4)*1024 .. +1024. K/V/weights replicated per core.
#
# Math (host-folded):
#   qn = (q * rstd_q) @ WqT_eff + bq/sqrt(hd)   with WqT_eff = diag(rms_q_w) Wq^T / sqrt(hd)
#   kn = (k * rstd_k) @ WkT_eff + bk            with WkT_eff = diag(rms_k_w) Wk^T
#   S_h = qn_h kn_h^T  (scale already folded into q side)
#   A   = sum_h exp(S_h) / rowsum_h(exp S_h)    (no max subtraction: |S| < 8)
#   out = A @ (0.25 * V)
#
# Structure notes (from trace analysis):
# - Dependencies coarsen to whole tiles through the semaphore encoding, so
#   every independently-consumed datum gets its OWN tile: kx/kxn/kT halves,
#   P per head, A/AT halves.  Slicing one big tile serializes consumers.
# - DVE rates: tensor_scalar 4x, tensor_tensor 2x, everything else 1x.
#   Head blend = TS-mul + TT-add tree; rmsnorm normalize = one broadcast TT.
# - ACT is the pacer (8 exp + 6 accumulator reads per 128-query block
#   ~10.9us); rowsum h0 rides a DVE tensor_reduce to balance (~10.8us).
# - ACT table sets: dummy sqrt at t=0 + dummy exp after the last real sqrt
#   -> exactly two table loads, both off the critical path.
import numpy as np

B, NQ, NK, D, DV = 2, 4096, 2048, 128, 768
H, HD = 4, 32
N_CORES = 8
NQC = NQ * B // N_CORES  # 1024 queries per core
NQT = NQC // 128  # 8 query tiles per core
NKT = NK // 128  # 16 key tiles
RMS_EPS = 1.1920929e-07

_CACHE = {}


def _build_nc():
    import concourse.bacc as bacc
    import concourse.mybir as mybir
    import concourse.tile as tile

    fp32 = mybir.dt.float32
    f16 = mybir.dt.float16

    nc = bacc.Bacc("TRN2", target_bir_lowering=False, debug=False)

    q_d = nc.dram_tensor("q", [NQC, D], f16, kind="ExternalInput").ap()
    k_d = nc.dram_tensor("k", [NK, D], f16, kind="ExternalInput").ap()
    v_d = nc.dram_tensor("v", [NK, DV], f16, kind="ExternalInput").ap()
    wq_d = nc.dram_tensor("wqt", [D, D], f16, kind="ExternalInput").ap()
    wk_d = nc.dram_tensor("wkt", [D, D], f16, kind="ExternalInput").ap()
    bq_d = nc.dram_tensor("bqe", [D], fp32, kind="ExternalInput").ap()
    bk_d = nc.dram_tensor("bke", [D], fp32, kind="ExternalInput").ap()
    o_d = nc.dram_tensor("o", [NQC, DV], fp32, kind="ExternalOutput").ap()

    with tile.TileContext(nc) as tc:
        _tile_kernel(tc, o_d, q_d, k_d, v_d, wq_d, wk_d, bq_d, bk_d)
    nc.compile()
    return nc


def _tile_kernel(tc, o_d, q_d, k_d, v_d, wq_d, wk_d, bq_d, bk_d):
    from contextlib import ExitStack

    import concourse.mybir as mybir

    nc = tc.nc
    fp32 = mybir.dt.float32
    f16 = mybir.dt.float16
    AF = mybir.ActivationFunctionType
    OP = mybir.AluOpType
    AX = mybir.AxisListType

    ctx = ExitStack()
    with ctx:
        singles = ctx.enter_context(tc.tile_pool(name="singles", bufs=1))

        # --- tiny constants via gpsimd (keeps DVE free for the preamble) ---
        junk = singles.tile([128, 512], f16)
        nc.gpsimd.memset(junk, 0.013)
        eps_sb = singles.tile([128, 1], fp32)
        nc.gpsimd.memset(eps_sb, RMS_EPS)
        dmy_in = singles.tile([128, 1], fp32)
        nc.gpsimd.memset(dmy_in, 0.5)
        dmy_out = singles.tile([128, 1], fp32)
        # dummy sqrt at t=0: loads the sqrt ACT table set while the input
        # DMAs are still in flight, so the real sqrts run immediately.
        nc.scalar.activation(dmy_out, dmy_in, AF.Sqrt)

        # --- input loads; k half0 first (it gates the first scores), v last
        # on the scalar dispatcher (first needed ~15us in).
        kx0 = singles.tile([128, 8, D], f16)
        kx1 = singles.tile([128, 8, D], f16)
        kv = k_d.rearrange("(p c) d -> p c d", c=NKT)
        nc.sync.dma_start(out=kx0, in_=kv[:, 0:8, :])
        qx_sb = singles.tile([128, NQT, D], f16)
        nc.sync.dma_start(out=qx_sb, in_=q_d.rearrange("(p c) d -> p c d", c=NQT))
        nc.sync.dma_start(out=kx1, in_=kv[:, 8:16, :])
        wq_sb = singles.tile([128, D], f16)
        nc.scalar.dma_start(out=wq_sb, in_=wq_d)
        wk_sb = singles.tile([128, D], f16)
        nc.scalar.dma_start(out=wk_sb, in_=wk_d)
        bq_sb = singles.tile([128, 1], fp32)
        nc.scalar.dma_start(out=bq_sb, in_=bq_d[:, None])
        bk_sb = singles.tile([128, 1], fp32)
        nc.scalar.dma_start(out=bk_sb, in_=bk_d[:, None])
        v_sb = singles.tile([128, NKT, DV], f16)
        vv = v_d.rearrange("(p c) d -> p c d", c=NKT)
        nc.scalar.dma_start(out=v_sb[:, 0:8, :], in_=vv[:, 0:8, :])
        nc.scalar.dma_start(out=v_sb[:, 8:16, :], in_=vv[:, 8:16, :])

        kxT0 = singles.tile([128, 1024], f16)  # normalized, transposed [d,tok]
        kxT1 = singles.tile([128, 1024], f16)
        qxT = singles.tile([128, NQC], f16)
        kT0 = singles.tile([128, 1024], f16)  # projected, head h rows 32h..
        kT1 = singles.tile([128, 1024], f16)
        qT = singles.tile([128, NQC], f16)
        # apath scratch
        t0 = singles.tile([128, NK], f16)
        t1 = singles.tile([128, NK], f16)
        t2 = singles.tile([128, NK], f16)
        t3 = singles.tile([128, NK], f16)

        spool = ctx.enter_context(tc.tile_pool(name="spsum", bufs=3, space="PSUM"))
        xpool = ctx.enter_context(tc.tile_pool(name="xpsum", bufs=1, space="PSUM"))
        pwork = ctx.enter_context(tc.tile_pool(name="pwork", bufs=2))
        awork = ctx.enter_context(tc.tile_pool(name="awork", bufs=2))
        owork = ctx.enter_context(tc.tile_pool(name="owork", bufs=3))
        small = ctx.enter_context(tc.tile_pool(name="small", bufs=2))
        pre = ctx.enter_context(tc.tile_pool(name="pre", bufs=1))

        # --- PE warm-up: junk MMs (~4us cold) release the HAM clock gate
        # (1.2 -> 2.4 GHz) before the first projection.
        warm = xpool.tile([128, 1024], fp32, tag="O", name="warm")
        for _ in range(12):
            nc.tensor.matmul(
                warm[:, 0:512], lhsT=junk[:, 0:128], rhs=junk,
                start=True, stop=True,
            )

        # ---------------- preamble: rmsnorm + transpose + projections -------
        def rms_side(x_sb, xn, sd, rstd, tag):
            # DVE: square (TT 2x) + reduce; ACT sqrt; DVE recip + one
            # broadcast-TT normalize for the whole 8-tile side.
            sq = pre.tile([128, 8, D], f16, tag="sq", bufs=2, name=f"sq_{tag}")
            nc.vector.tensor_mul(sq, x_sb, x_sb)
            ssq = pre.tile([128, 8], fp32, tag=f"ssq_{tag}", name=f"ssq_{tag}")
            nc.vector.tensor_reduce(ssq[:, :, None], sq, AX.X, OP.add)
            nc.scalar.activation(sd, ssq, AF.Sqrt, bias=eps_sb, scale=1.0 / D)
            nc.vector.reciprocal(rstd, sd)
            nc.vector.tensor_tensor(
                xn, x_sb, rstd[:, :, None].broadcast_to([128, 8, D]), OP.mult
            )

        def transpose_side(xn, xT):
            nc.sync.dma_start_transpose(
                out=xT.rearrange("p (c j) -> p c j", j=128),
                in_=xn.rearrange("p c j -> p (c j)"),
            )

        _pj = [0]

        def proj_chunk(xT, j, w_sb, b_sb, dst, dj, bias_eng):
            # psum rides the (idle) S pool; one tile per 512-chunk so the
            # next chunk's MM never waits on this chunk's bias eviction.
            _pj[0] += 1
            pp = spool.tile([128, 1024], fp32, tag="S", name=f"pp{_pj[0]}")
            nc.tensor.matmul(
                pp[:, 0:512], lhsT=w_sb, rhs=xT[:, j * 512 : (j + 1) * 512],
                start=True, stop=True,
            )
            dsl = slice(dj * 512, (dj + 1) * 512)
            if bias_eng == "act":
                nc.scalar.add(dst[:, dsl], pp[:, 0:512], b_sb)
            else:
                nc.vector.tensor_scalar(dst[:, dsl], pp[:, 0:512], b_sb, None, OP.add)

        kxn0 = pre.tile([128, 8, D], f16, tag="kxn0", name="kxn0")
        kxn1 = pre.tile([128, 8, D], f16, tag="kxn1", name="kxn1")
        qxn = pre.tile([128, NQT, D], f16, tag="qxn", name="qxn")
        ksd0 = pre.tile([128, 8], fp32, tag="ksd0", name="ksd0")
        ksd1 = pre.tile([128, 8], fp32, tag="ksd1", name="ksd1")
        qsd = pre.tile([128, NQT], fp32, tag="qsd", name="qsd")
        krstd0 = pre.tile([128, 8], fp32, tag="krstd0", name="krstd0")
        krstd1 = pre.tile([128, 8], fp32, tag="krstd1", name="krstd1")
        qrstd = pre.tile([128, NQT], fp32, tag="qrstd", name="qrstd")

        rms_side(kx0, kxn0, ksd0, krstd0, "k0")
        transpose_side(kxn0, kxT0)
        proj_chunk(kxT0, 0, wk_sb, bk_sb, kT0, 0, "act")
        proj_chunk(kxT0, 1, wk_sb, bk_sb, kT0, 1, "act")
        rms_side(qx_sb, qxn, qsd, qrstd, "q")
        transpose_side(qxn, qxT)
        proj_chunk(qxT, 0, wq_sb, bq_sb, qT, 0, "act")
        proj_chunk(qxT, 1, wq_sb, bq_sb, qT, 1, "act")
        rms_side(kx1, kxn1, ksd1, krstd1, "k1")
        # dummy exp AFTER the last sqrt: prefetches the exp table set once.
        nc.scalar.activation(dmy_out, dmy_in, AF.Exp)
        transpose_side(kxn1, kxT1)
        # k1 bias on DVE: ACT is about to start the block-0 exps.
        proj_chunk(kxT1, 0, wk_sb, bk_sb, kT1, 0, "dve")
        proj_chunk(kxT1, 1, wk_sb, bk_sb, kT1, 1, "dve")

        # ---------------- software-pipelined main loop -----------------------
        st = {}

        def emit_scores_pair(qc, pair):
            # 2 heads' score MMs issued interleaved -> concurrent execution on
            # disjoint 32-row strips of the PE array (row-group tiling).
            qsl = slice(qc * 128, (qc + 1) * 128)
            hA, hB = 2 * pair, 2 * pair + 1
            s = st[qc]
            for half in (0, 1):
                for h in (hA, hB):
                    s[f"S{h}{half}"] = spool.tile(
                        [128, 1024], fp32, tag="S", name=f"S_{qc}_{h}_{half}"
                    )
            for kc in range(4):
                half, sub = divmod(kc, 2)
                kTh = (kT0, kT1)[half]
                for h in (hA, hB):
                    nc.tensor.matmul(
                        s[f"S{h}{half}"][:, sub * 512 : (sub + 1) * 512],
                        lhsT=qT[32 * h : 32 * (h + 1), qsl],
                        rhs=kTh[32 * h : 32 * (h + 1), sub * 512 : (sub + 1) * 512],
                        start=True,
                        stop=True,
                        tile_position=(32 * h, 0),
                    )

        def emit_exp(qc, h, half):
            s = st[qc]
            kwargs = {}
            if h >= 1:  # h1..h3 rowsums ride the exp accumulator
                kwargs["accum_out"] = s["racc"][:, h - 1, half : half + 1]
            nc.scalar.activation(
                s[f"P{h}"][:, half * 1024 : (half + 1) * 1024],
                s[f"S{h}{half}"],
                AF.Exp,
                **kwargs,
            )

        def emit_rsum_dve(qc, h):
            s = st[qc]
            nc.vector.tensor_reduce(
                s["rsum"][:, h : h + 1], s[f"P{h}"], AX.X, OP.add
            )

        def emit_racc_merge(qc):
            s = st[qc]
            nc.vector.tensor_tensor(
                s["rsum"][:, 1:4], s["racc"][:, :, 0], s["racc"][:, :, 1], OP.add
            )

        def emit_recip(qc):
            s = st[qc]
            nc.vector.reciprocal(s["crec"], s["rsum"])

        def emit_ap_half(qc, hh):
            # blend tree on TS(4x)/TT(2x): A = (P0c0+P1c1)+(P2c2+P3c3)
            s = st[qc]
            crec = s["crec"]
            sl = slice(hh * 1024, (hh + 1) * 1024)
            A = s[f"A{hh}"]
            nc.vector.tensor_scalar_mul(t0[:, sl], s["P0"][:, sl], crec[:, 0:1])
            nc.vector.tensor_scalar_mul(t1[:, sl], s["P1"][:, sl], crec[:, 1:2])
            nc.vector.tensor_add(t0[:, sl], t0[:, sl], t1[:, sl])
            nc.vector.tensor_scalar_mul(t2[:, sl], s["P2"][:, sl], crec[:, 2:3])
            nc.vector.tensor_scalar_mul(t3[:, sl], s["P3"][:, sl], crec[:, 3:4])
            nc.vector.tensor_add(t2[:, sl], t2[:, sl], t3[:, sl])
            nc.vector.tensor_add(A, t0[:, sl], t2[:, sl])

        def emit_at(qc, hh):
            s = st[qc]
            nc.sync.dma_start_transpose(
                out=s[f"AT{hh}"].rearrange("p (c j) -> p c j", j=128),
                in_=s[f"A{hh}"],
            )

        def emit_pv(qc, kcs):
            s = st[qc]
            for kc in kcs:
                AT = s[f"AT{kc // 8}"]
                for dvh in (0, 1):
                    nc.tensor.matmul(
                        s["O"][:, dvh * 512 : dvh * 512 + 384],
                        lhsT=AT[:, (kc % 8) * 128 : (kc % 8 + 1) * 128],
                        rhs=v_sb[:, kc, dvh * 384 : (dvh + 1) * 384],
                        start=kc == 0,
                        stop=kc == NKT - 1,
                    )

        def emit_evict(qc):
            s = st[qc]
            nc.vector.tensor_copy(
                s["osb"],
                s["O"].rearrange("p (c x) -> p c x", c=2)[:, :, 0:384],
            )

        o_view = o_d.rearrange("(j c) d -> c j d", c=NQT)

        def emit_out(qc):
            # halved across two dispatchers: keeps the sync hw queue light so
            # A-transposes aren't stuck behind 384KB output transfers.
            osb = st[qc]["osb"].rearrange("p c x -> p (c x)")
            nc.sync.dma_start(out=o_view[qc][:, 0:384], in_=osb[:, 0:384])
            nc.scalar.dma_start(out=o_view[qc][:, 384:768], in_=osb[:, 384:768])

        def alloc_block(qc):
            st[qc] = {
                "racc": small.tile([128, 3, 2], fp32, tag="racc", name=f"ra_{qc}"),
                "rsum": small.tile([128, H], fp32, tag="rsum", name=f"rs_{qc}"),
                "crec": small.tile([128, H], fp32, tag="crec", name=f"cr_{qc}"),
                "osb": owork.tile([128, 2, 384], fp32, tag="osb", name=f"osb_{qc}"),
            }
            for h in range(H):
                st[qc][f"P{h}"] = pwork.tile(
                    [128, NK], f16, tag=f"P{h}", name=f"P{h}_{qc}"
                )
            for hh in (0, 1):
                st[qc][f"A{hh}"] = awork.tile(
                    [128, 1024], f16, tag=f"A{hh}", name=f"A{hh}_{qc}"
                )
                st[qc][f"AT{hh}"] = awork.tile(
                    [128, 1024], f16, tag=f"AT{hh}", name=f"AT{hh}_{qc}"
                )

        last_o = [warm]

        def alloc_o(qc):
            st[qc]["O"] = xpool.tile([128, 1024], fp32, tag="O", name=f"O_{qc}")
            last_o[0] = st[qc]["O"]

        _fid = [0]

        def emit_fillers(n):
            # HAM keep-warm: dep-free junk MMs into the O tile's dead columns
            # (PV uses [0:384] and [512:896] only).  Accumulated PE micro-
            # idles re-throttle the clock gate (1.2 GHz) without these.
            return
            for _ in range(n):
                _fid[0] += 1
                nc.tensor.matmul(
                    last_o[0][:, 896:1024], lhsT=junk[:, 0:128],
                    rhs=junk[:, 0:128], start=True, stop=True,
                )

        for qc in range(NQT):
            alloc_block(qc)
            # PE queue per block: scores-p0 | PV(qc-2) kc8-15 | scores-p1 |
            # PV(qc-1) kc0-7.  PV parts straddle blocks so score MMs are never
            # head-of-line blocked behind a PV burst waiting on a transpose.
            emit_scores_pair(qc, 0)
            emit_fillers(6)
            if qc >= 2:
                emit_pv(qc - 2, range(8, NKT))
            emit_fillers(6)
            # DVE stream: ap(qc-1)-h0 | evict(qc-2) | rsum(qc,0) | ap(qc-1)-h1
            # | racc-merge | recip  (every instr's data ready before the queue
            # reaches it).
            if qc >= 1:
                emit_ap_half(qc - 1, 0)
                emit_at(qc - 1, 0)
            emit_exp(qc, 0, 0)
            emit_exp(qc, 1, 0)
            emit_exp(qc, 0, 1)
            emit_exp(qc, 1, 1)
            if qc >= 2:
                emit_evict(qc - 2)
                emit_out(qc - 2)
            emit_rsum_dve(qc, 0)
            if qc >= 1:
                emit_ap_half(qc - 1, 1)
                emit_at(qc - 1, 1)
            emit_scores_pair(qc, 1)
            if qc >= 1:
                alloc_o(qc - 1)
                emit_pv(qc - 1, range(0, 8))
            emit_fillers(8)
            emit_exp(qc, 2, 0)
            emit_exp(qc, 3, 0)
            emit_exp(qc, 2, 1)
            emit_exp(qc, 3, 1)
            emit_racc_merge(qc)
            emit_recip(qc)
            if qc == 0:
                emit_fillers(10)

        # ---------------- drain ----------------------------------------------
        qc = NQT - 1
        emit_pv(qc - 1, range(8, NKT))
        emit_fillers(10)
        emit_ap_half(qc, 0)
        emit_at(qc, 0)
        emit_evict(qc - 1)
        emit_out(qc - 1)
        alloc_o(qc)
        emit_pv(qc, range(0, 8))
        emit_fillers(8)
        emit_ap_half(qc, 1)
        emit_at(qc, 1)
        emit_pv(qc, range(8, NKT))
        emit_evict(qc)
        emit_out(qc)


def _get_nc():
    if "nc" not in _CACHE:
        _CACHE["nc"] = _build_nc()
    return _CACHE["nc"]


def _host_prep(query, key, value, rms_q_w, rms_k_w, Wq, Wk, bq, bk):
    s = np.sqrt(float(HD))
    wqt = (rms_q_w[:, None] * Wq.T / s).astype(np.float16)
    wkt = (rms_k_w[:, None] * Wk.T).astype(np.float16)
    bqe = (bq / s).astype(np.float32)
    bke = bk.astype(np.float32)
    vq = (0.25 * value).astype(np.float16)  # [B, NK, DV]
    in_maps = []
    nq_blk = NQ // (N_CORES // B)  # 1024
    for c in range(N_CORES):
        b, qi = divmod(c, N_CORES // B)
        in_maps.append(
            {
                "q": np.ascontiguousarray(
                    query[b, qi * nq_blk : (qi + 1) * nq_blk]
                ).astype(np.float16),
                "k": np.ascontiguousarray(key[b]).astype(np.float16),
                "v": np.ascontiguousarray(vq[b]),
                "wqt": wqt,
                "wkt": wkt,
                "bqe": bqe,
                "bke": bke,
            }
        )
    return in_maps


def kernel(query, key, value, rms_q_w, rms_k_w, Wq, Wk, bq, bk, _trace=False):
    from concourse import bass_utils

    in_maps = _host_prep(
        np.asarray(query), np.asarray(key), np.asarray(value),
        np.asarray(rms_q_w), np.asarray(rms_k_w),
        np.asarray(Wq), np.asarray(Wk), np.asarray(bq), np.asarray(bk),
    )
    nc = _get_nc()
    res = bass_utils.run_bass_kernel_spmd(
        nc, in_maps, core_ids=list(range(N_CORES)), trace=_trace
    )
    _CACHE["last_results"] = res
    outs = [np.asarray(r["o"], dtype=np.float32) for r in res.results]
    nq_blk = NQ // (N_CORES // B)
    out = np.empty((B, NQ, DV), dtype=np.float32)
    for c in range(N_CORES):
        b, qi = divmod(c, N_CORES // B)
        out[b, qi * nq_blk : (qi + 1) * nq_blk] = outs[c]
    return out
